# revision 1
# baseline (speedup 1.0000x reference)
"""Causal self-attention (B=2, T=2048, C=1024, H=16) on 8 TRN2 NeuronCores.

Sharding: data-parallel over batch (2 groups of 4 cores) x tensor-parallel
over heads (4 heads per core, Megatron-style column/row split of the
qkv / proj weights). Each core computes, for its (batch, head-group):

    qkT/kT  = (W_qk^T x^T + b_qk)        [512, 2048]   (transposed layout)
    v       = x W_v + b_v                [2048, 256]
    S^T     = kT^T-tiles x qT            per (head, i-chunk) blocks
    P       = exp(S^T / 8) * causal_mask (no max-subtraction: logits are
                                          small, |S/8| < ~3, exp is safe)
    Yu^T    = [v | 1]^T P^T              rows 0..63 unnormalized y^T,
                                          row 64 = softmax denominator
    y^T     = Yu^T * (1/denom)           (denominator broadcast across
                                          partitions via a K=1 matmul)
    out^T  += W_p-rows^T y^T             [1024, 2048] partial projection

The host sums the 4 partial projections per batch and adds b_proj.

All matmuls run as float32r (full-rate fp32 streaming on the PE array,
~1.5e-4 relative error vs fp32; plain fp32 matmul is 4x slower).
"""

import numpy as np

B, T, C, H = 2, 2048, 1024, 16
HD = C // H  # 64
HG = 4  # head-groups (tensor-parallel degree); B * HG = 8 cores
HPG = H // HG  # heads per group = 4
DG = HPG * HD  # columns per group = 256
TC = 512  # t-chunk (moving free dim)
NTC = T // TC  # 4
NJT = T // 128  # 16 j-tiles of 128 keys

_cached = {}


def _build():
    import concourse.mybir as mybir
    import concourse.tile as tile
    from concourse import bacc

    F32 = mybir.dt.float32
    F32R = mybir.dt.float32r
    Act = mybir.ActivationFunctionType

    nc = bacc.Bacc()
    x_d = nc.declare_dram_parameter("xt", [C, T], F32R, isOutput=False)
    wqk_d = nc.declare_dram_parameter("wqk", [C, 2 * DG], F32R, isOutput=False)
    bqk_d = nc.declare_dram_parameter("bqk", [2 * DG, 1], F32, isOutput=False)
    wv_d = nc.declare_dram_parameter("wv", [C, DG], F32R, isOutput=False)
    bv_d = nc.declare_dram_parameter("bv", [1, DG], F32, isOutput=False)
    wp_d = nc.declare_dram_parameter("wp", [DG, C], F32R, isOutput=False)
    out_d = nc.declare_dram_parameter("outt", [C, T], F32, isOutput=True)

    with tile.TileContext(nc) as tc:
        with (
            tc.tile_pool(name="const", bufs=1) as const,
            tc.tile_pool(name="sb", bufs=1) as sb,
            tc.tile_pool(name="ps", bufs=1, space="PSUM") as ps,
        ):
            # ---- constants ----
            # causal masks for the 4 diagonal sub-blocks of an i-chunk:
            # mask_k[j, i] = 1 iff i - 128*k - j >= 0
            masks = []
            mask_f = const.tile([128, TC], F32)
            for k in range(4):
                nc.vector.memset(mask_f, 1.0)
                nc.gpsimd.affine_select(
                    out=mask_f,
                    in_=mask_f,
                    compare_op=mybir.AluOpType.is_ge,
                    fill=0.0,
                    base=-128 * k,
                    pattern=[[1, TC]],
                    channel_multiplier=-1,
                )
                mk = const.tile([128, TC], F32R, name=f"mask{k}", uniquify=True)
                nc.vector.tensor_copy(mk, mask_f)
                masks.append(mk)

            ones_f = const.tile([128, 1], F32)
            nc.vector.memset(ones_f, 1.0)
            ones_r = const.tile([1, HD], F32R)
            nc.vector.tensor_copy(ones_r, ones_f[0:1, :].broadcast_to((1, HD)))

            # x arrives pre-transposed from the host: [C, T], c on partitions
            def build_xt(tcx):
                xt = sb.tile([128, 8, TC], F32R, tag="xt", bufs=2, name=f"xt{tcx}")
                for kc in range(8):
                    nc.sync.dma_start(
                        out=xt[:, kc, :],
                        in_=x_d[
                            kc * 128 : (kc + 1) * 128,
                            tcx * TC : (tcx + 1) * TC,
                        ],
                    )
                return xt

            xt = build_xt(0)

            # ---- weights ----
            wqk_sb = const.tile([128, 8, 2 * DG], F32R)
            wv_sb = const.tile([128, 8, DG], F32R)
            wp_sb = const.tile([128, 2, C], F32R)
            bqk_sb = const.tile([128, 4], F32)
            bv_sb = const.tile([128, DG], F32)
            for kc in range(8):
                nc.sync.dma_start(
                    out=wqk_sb[:, kc, :], in_=wqk_d[kc * 128 : (kc + 1) * 128, :]
                )
                nc.sync.dma_start(
                    out=wv_sb[:, kc, :], in_=wv_d[kc * 128 : (kc + 1) * 128, :]
                )
            for cc in range(2):
                nc.sync.dma_start(
                    out=wp_sb[:, cc, :], in_=wp_d[cc * 128 : (cc + 1) * 128, :]
                )
            for jt in range(4):
                nc.sync.dma_start(
                    out=bqk_sb[:, jt : jt + 1],
                    in_=bqk_d[jt * 128 : (jt + 1) * 128, :],
                )
            nc.gpsimd.dma_start(out=bv_sb, in_=bv_d[:].to_broadcast((128, DG)))

            # ---- persistent activations ----
            # qkT [j, t]: j-tiles 0,1 = q (heads 0..3), 2,3 = k
            qk_sb = const.tile([128, 4, T], F32R)
            # v1 [t, d]: per (t-tile, head): 64 v columns + ones column
            v1_sb = const.tile([128, NJT, HPG, HD + 1], F32R)
            for tt in range(NJT):
                for h in range(HPG):
                    nc.vector.tensor_copy(v1_sb[:, tt, h, HD : HD + 1], ones_f)
            # y^T [c', t]: c' = head-major 256 rows in 2 tiles
            yt_sb = const.tile([128, 2, T], F32R)

            for tcx in range(NTC):
                # ---- phase 1: qkT, v from the pre-built transposed chunk ----
                for jt in range(4):
                    pqk = ps.tile([128, TC], F32, tag="mm512", bufs=4)
                    for kc in range(8):
                        nc.tensor.matmul(
                            pqk,
                            wqk_sb[:, kc, jt * 128 : (jt + 1) * 128],
                            xt[:, kc, :],
                            start=(kc == 0),
                            stop=(kc == 7),
                        )
                    nc.scalar.activation(
                        qk_sb[:, jt, tcx * TC : (tcx + 1) * TC],
                        pqk,
                        Act.Identity,
                        bias=bqk_sb[:, jt : jt + 1],
                    )
                for tt in range(4):
                    pv = ps.tile([128, DG], F32, tag="v256", bufs=1)
                    for kc in range(8):
                        nc.tensor.matmul(
                            pv,
                            xt[:, kc, tt * 128 : (tt + 1) * 128],
                            wv_sb[:, kc, :],
                            start=(kc == 0),
                            stop=(kc == 7),
                        )
                    for h in range(HPG):
                        nc.vector.tensor_add(
                            v1_sb[:, tcx * 4 + tt, h, 0:HD],
                            pv[:, h * HD : (h + 1) * HD],
                            bv_sb[:, h * HD : (h + 1) * HD],
                        )

                # ---- phase 2: attention for i-chunk q = tcx, all heads ----
                q = tcx
                njt = 4 * (q + 1)
                for h in range(HPG):
                    py = ps.tile([HD + 1, TC], F32, tag="py", bufs=2)
                    # software-pipelined: emit S(jt+2) before Y(jt) so the PE
                    # never waits on the exp/mask of the block it just scored
                    pending = []  # (jt, cut, p_sb) awaiting their Y matmul
                    for jt in range(njt):
                        # diagonal j-tiles: columns i < 128k are fully masked;
                        # narrow to i >= cut (256-multiple so f32r stays fast)
                        prow = (h % 2) * 64
                        k = jt - 4 * q
                        cut = 0 if k <= 0 else (128 if k == 1 else 256)
                        sp = ps.tile([128, TC], F32, tag="mm512", bufs=4)
                        nc.tensor.matmul(
                            sp[:, cut:],
                            qk_sb[
                                prow : prow + 64,
                                2 + h // 2,
                                jt * 128 : (jt + 1) * 128,
                            ],
                            qk_sb[
                                prow : prow + 64,
                                h // 2,
                                q * TC + cut : (q + 1) * TC,
                            ],
                            start=True,
                            stop=True,
                        )
                        p_sb = sb.tile([128, TC], F32R, tag="p", bufs=8)
                        nc.scalar.activation(
                            p_sb[:, cut:], sp[:, cut:], Act.Exp, scale=0.125
                        )
                        if k >= 0:
                            nc.vector.tensor_mul(
                                p_sb[:, cut:], p_sb[:, cut:], masks[k][:, cut:]
                            )
                        pending.append((jt, cut, p_sb))
                        if len(pending) > 3:
                            pjt, pcut, pp = pending.pop(0)
                            nc.tensor.matmul(
                                py[:, pcut:],
                                v1_sb[:, pjt, h, :],
                                pp[:, pcut:],
                                start=(pjt == 0),
                                stop=False,
                            )
                    for pjt, pcut, pp in pending:
                        nc.tensor.matmul(
                            py[:, pcut:],
                            v1_sb[:, pjt, h, :],
                            pp[:, pcut:],
                            start=(pjt == 0),
                            stop=(pjt == njt - 1),
                        )
                    dn_sb = sb.tile([1, TC], F32, tag="dn_sb", bufs=2)
                    nc.scalar.activation(dn_sb, py[HD : HD + 1, :], Act.Copy)
                    recip_f = sb.tile([1, TC], F32, tag="recip_f", bufs=2)
                    nc.vector.reciprocal_approx_fast(recip_f, dn_sb)
                    recip = sb.tile([1, TC], F32R, tag="recip", bufs=2)
                    nc.vector.tensor_copy(recip, recip_f)
                    yu_sb = sb.tile([HD, TC], F32, tag="yu", bufs=2)
                    nc.scalar.activation(yu_sb, py[0:HD, :], Act.Copy)
                    bp = ps.tile([HD, TC], F32, tag="bp", bufs=1)
                    nc.tensor.matmul(bp, ones_r, recip, start=True, stop=True)
                    nc.vector.tensor_mul(
                        yt_sb[prow : prow + 64, h // 2, q * TC : (q + 1) * TC],
                        yu_sb,
                        bp,
                    )

                # build next chunk's x^T now: its transposes fill the PE gap
                # left by the last head's normalization chain
                if tcx + 1 < NTC:
                    xt = build_xt(tcx + 1)

                # ---- phase 3: partial projection for t-chunk = tcx ----
                for mt in range(8):
                    po = ps.tile([128, TC], F32, tag="mm512", bufs=4)
                    for cc in range(2):
                        nc.tensor.matmul(
                            po,
                            wp_sb[:, cc, mt * 128 : (mt + 1) * 128],
                            yt_sb[:, cc, tcx * TC : (tcx + 1) * TC],
                            start=(cc == 0),
                            stop=(cc == 1),
                        )
                    ot = sb.tile([128, TC], F32, tag="ot", bufs=3)
                    nc.vector.tensor_copy(ot, po)
                    nc.sync.dma_start(
                        out=out_d[
                            mt * 128 : (mt + 1) * 128, tcx * TC : (tcx + 1) * TC
                        ],
                        in_=ot,
                    )

    nc.finalize()
    return nc


def _in_maps(x, W_attn, b_attn, W_proj):
    in_maps = []
    for core in range(8):
        b = core // HG
        hg = core % HG
        qs, ks, vs = hg * DG, C + hg * DG, 2 * C + hg * DG
        wqk = np.concatenate(
            [W_attn[:, qs : qs + DG], W_attn[:, ks : ks + DG]], axis=1
        )
        bqk = np.concatenate(
            [b_attn[qs : qs + DG], b_attn[ks : ks + DG]]
        ).reshape(2 * DG, 1)
        in_maps.append(
            {
                "xt": np.ascontiguousarray(x[b].T),
                "wqk": np.ascontiguousarray(wqk),
                "bqk": np.ascontiguousarray(bqk),
                "wv": np.ascontiguousarray(W_attn[:, vs : vs + DG]),
                "bv": np.ascontiguousarray(b_attn[vs : vs + DG].reshape(1, DG)),
                "wp": np.ascontiguousarray(W_proj[hg * DG : (hg + 1) * DG, :]),
            }
        )
    return in_maps


def _combine(results, b_proj):
    out = np.empty((B, T, C), dtype=np.float32)
    for b in range(B):
        acc = results[4 * b]["outt"].astype(np.float32)
        for hg in range(1, HG):
            acc = acc + results[4 * b + hg]["outt"]
        out[b] = acc.T + b_proj
    return out


def get_nc():
    if "nc" not in _cached:
        _cached["nc"] = _build()
    return _cached["nc"]


def kernel(x, W_attn, b_attn, W_proj, b_proj):
    from concourse.bass_utils import run_bass_kernel_spmd

    nc = get_nc()
    x = np.asarray(x, dtype=np.float32)
    W_attn = np.asarray(W_attn, dtype=np.float32)
    b_attn = np.asarray(b_attn, dtype=np.float32)
    W_proj = np.asarray(W_proj, dtype=np.float32)
    b_proj = np.asarray(b_proj, dtype=np.float32)

    in_maps = _in_maps(x, W_attn, b_attn, W_proj)
    r = run_bass_kernel_spmd(nc, in_maps, core_ids=list(range(8)), trace=False)
    return _combine(r.results, b_proj)



# revision 9
# speedup vs baseline: 1.4304x; 1.4304x over previous
"""Causal self-attention (B=2, T=2048, C=1024, H=16) on 8 TRN2 NeuronCores.

Sharding: data-parallel over batch (2 groups of 4 cores) x tensor-parallel
over heads (4 heads per core, Megatron-style column/row split of the
qkv / proj weights). Each core computes, for its (batch, head-group):

    qkT/kT  = (W_qk^T x^T + b_qk)        [512, 2048]   (transposed layout)
    v       = x W_v + b_v                [2048, 256]
    S^T     = kT^T-tiles x qT            per (head, i-chunk) blocks
    P       = exp(S^T / 8) * causal_mask (no max-subtraction: logits are
                                          small, |S/8| < ~3, exp is safe)
    Yu^T    = [v | 1]^T P^T              rows 0..63 unnormalized y^T,
                                          row 64 = softmax denominator
    y^T     = Yu^T * (1/denom)           (denominator broadcast across
                                          partitions via a K=1 matmul)
    out^T  += W_p-rows^T y^T             [1024, 2048] partial projection

The host sums the 4 partial projections per batch and adds b_proj.

All matmuls run in bf16 (full-rate streaming, prefetchable LDWEIGHTS;
fp32r's in-matmul weight load serializes and runs ~2.6 cyc/col on the
attention blocks). PSUM accumulation stays fp32. Scalar engine runs ONLY
the softmax Exp; bias adds / copies live on the vector engine.
"""

import numpy as np

B, T, C, H = 2, 2048, 1024, 16
HD = C // H  # 64
HG = 4  # head-groups (tensor-parallel degree); B * HG = 8 cores
HPG = H // HG  # heads per group = 4
DG = HPG * HD  # columns per group = 256
TC = 512  # t-chunk (moving free dim)
NTC = T // TC  # 4
NJT = T // 128  # 16 j-tiles of 128 keys

_cached = {}


def _build():
    import concourse.mybir as mybir
    import concourse.tile as tile
    from concourse import bacc

    F32 = mybir.dt.float32
    F32R = mybir.dt.float32r
    BF16 = mybir.dt.bfloat16
    Act = mybir.ActivationFunctionType

    nc = bacc.Bacc()
    x_d = nc.declare_dram_parameter("xt", [C, T], BF16, isOutput=False)
    wqk_d = nc.declare_dram_parameter("wqk", [C, 2 * DG], BF16, isOutput=False)
    bqk_d = nc.declare_dram_parameter("bqk", [2 * DG, 1], F32, isOutput=False)
    wv_d = nc.declare_dram_parameter("wv", [C, DG], BF16, isOutput=False)
    bv_d = nc.declare_dram_parameter("bv", [1, DG], F32, isOutput=False)
    wp_d = nc.declare_dram_parameter("wp", [DG, C], BF16, isOutput=False)
    out_d = nc.declare_dram_parameter("outt", [C, T], BF16, isOutput=True)

    with tile.TileContext(nc) as tc:
        with (
            tc.tile_pool(name="const", bufs=1) as const,
            tc.tile_pool(name="sb", bufs=1) as sb,
            tc.tile_pool(name="ps", bufs=1, space="PSUM") as ps,
        ):
            # ---- constants ----
            # causal masks for the 4 diagonal sub-blocks of an i-chunk:
            # mask_k[j, i] = 1 iff i - 128*k - j >= 0
            masks = []
            mask_f = const.tile([128, TC], F32)
            for k in range(4):
                nc.vector.memset(mask_f, 1.0)
                nc.gpsimd.affine_select(
                    out=mask_f,
                    in_=mask_f,
                    compare_op=mybir.AluOpType.is_ge,
                    fill=0.0,
                    base=-128 * k,
                    pattern=[[1, TC]],
                    channel_multiplier=-1,
                )
                mk = const.tile([128, TC], BF16, name=f"mask{k}", uniquify=True)
                nc.vector.tensor_copy(mk, mask_f)
                masks.append(mk)

            ones_f = const.tile([128, 1], F32)
            nc.vector.memset(ones_f, 1.0)

            # x arrives pre-transposed from the host: [C, T], c on partitions
            def build_xt(tcx):
                xt = sb.tile([128, 8, TC], BF16, tag="xt", bufs=2, name=f"xt{tcx}")
                for kc in range(8):
                    nc.sync.dma_start(
                        out=xt[:, kc, :],
                        in_=x_d[
                            kc * 128 : (kc + 1) * 128,
                            tcx * TC : (tcx + 1) * TC,
                        ],
                    )
                return xt

            xt = build_xt(0)

            # ---- weights ----
            wqk_sb = const.tile([128, 8, 2 * DG], BF16)
            wv_sb = const.tile([128, 8, DG], BF16)
            wp_sb = const.tile([128, 2, C], BF16)
            bqk_sb = const.tile([128, 4], F32)
            bv_sb = const.tile([128, HPG, HD], F32)
            for kc in range(8):
                nc.sync.dma_start(
                    out=wqk_sb[:, kc, :], in_=wqk_d[kc * 128 : (kc + 1) * 128, :]
                )
                nc.sync.dma_start(
                    out=wv_sb[:, kc, :], in_=wv_d[kc * 128 : (kc + 1) * 128, :]
                )
            for cc in range(2):
                nc.sync.dma_start(
                    out=wp_sb[:, cc, :], in_=wp_d[cc * 128 : (cc + 1) * 128, :]
                )
            for jt in range(4):
                nc.sync.dma_start(
                    out=bqk_sb[:, jt : jt + 1],
                    in_=bqk_d[jt * 128 : (jt + 1) * 128, :],
                )
            for h in range(HPG):
                nc.gpsimd.dma_start(
                    out=bv_sb[:, h, :],
                    in_=bv_d[0:1, h * HD : (h + 1) * HD].to_broadcast((128, HD)),
                )

            # ---- persistent activations ----
            # qkT [j, t]: j-tiles 0,1 = q (heads 0..3), 2,3 = k
            qk_sb = const.tile([128, 4, T], BF16)
            # v1 [t, d]: per (t-tile, head): 64 v columns + ones column
            v1_sb = const.tile([128, NJT, HPG, HD + 1], BF16)
            for tt in range(NJT):
                for h in range(HPG):
                    nc.vector.tensor_copy(v1_sb[:, tt, h, HD : HD + 1], ones_f)
            # y^T [c', t]: c' = head-major 256 rows in 2 tiles
            yt_sb = const.tile([128, 2, T], BF16)

            for tcx in range(NTC):
                # ---- phase 1: qkT, v from the pre-built transposed chunk ----
                for jt in range(4):
                    pqk = ps.tile([128, TC], F32, tag="mm512", bufs=4)
                    for kc in range(8):
                        nc.tensor.matmul(
                            pqk,
                            wqk_sb[:, kc, jt * 128 : (jt + 1) * 128],
                            xt[:, kc, :],
                            start=(kc == 0),
                            stop=(kc == 7),
                        )
                    nc.vector.tensor_scalar_add(
                        qk_sb[:, jt, tcx * TC : (tcx + 1) * TC],
                        pqk,
                        bqk_sb[:, jt : jt + 1],
                    )
                for tt in range(4):
                    pv = ps.tile([128, HPG, HD], F32, tag="v256", bufs=1)
                    for kc in range(8):
                        nc.tensor.matmul(
                            pv,
                            xt[:, kc, tt * 128 : (tt + 1) * 128],
                            wv_sb[:, kc, :],
                            start=(kc == 0),
                            stop=(kc == 7),
                        )
                    nc.vector.tensor_add(
                        v1_sb[:, tcx * 4 + tt, :, 0:HD], pv, bv_sb
                    )

                # ---- phase 2: attention for i-chunk q = tcx, all heads ----
                q = tcx
                njt = 4 * (q + 1)
                for h in range(HPG):
                    py = ps.tile([HD + 1, TC], F32, tag="py", bufs=2)
                    # software-pipelined: emit S(jt+2) before Y(jt) so the PE
                    # never waits on the exp/mask of the block it just scored
                    pending = []  # (jt, cut, p_sb) awaiting their Y matmul
                    for jt in range(njt):
                        # diagonal j-tiles: columns i < 128k are fully masked;
                        # narrow to i >= 128k
                        prow = (h % 2) * 64
                        k = jt - 4 * q
                        cut = 0 if k <= 0 else 128 * k
                        sp = ps.tile([128, TC], F32, tag="mm512", bufs=4)
                        nc.tensor.matmul(
                            sp[:, cut:],
                            qk_sb[
                                prow : prow + 64,
                                2 + h // 2,
                                jt * 128 : (jt + 1) * 128,
                            ],
                            qk_sb[
                                prow : prow + 64,
                                h // 2,
                                q * TC + cut : (q + 1) * TC,
                            ],
                            start=True,
                            stop=True,
                        )
                        p_sb = sb.tile([128, TC], BF16, tag="p", bufs=8)
                        nc.scalar.activation(
                            p_sb[:, cut:], sp[:, cut:], Act.Exp, scale=0.125
                        )
                        if k >= 0:
                            nc.vector.tensor_mul(
                                p_sb[:, cut:], p_sb[:, cut:], masks[k][:, cut:]
                            )
                        pending.append((jt, cut, p_sb))
                        if len(pending) > 3:
                            pjt, pcut, pp = pending.pop(0)
                            nc.tensor.matmul(
                                py[:, pcut:],
                                v1_sb[:, pjt, h, :],
                                pp[:, pcut:],
                                start=(pjt == 0),
                                stop=False,
                            )
                    for pjt, pcut, pp in pending:
                        nc.tensor.matmul(
                            py[:, pcut:],
                            v1_sb[:, pjt, h, :],
                            pp[:, pcut:],
                            start=(pjt == 0),
                            stop=(pjt == njt - 1),
                        )
                    # the custom-DVE reciprocal cannot read PSUM on hardware
                    # (CoreSim accepts it, HW returns garbage) — stage the
                    # denominator row through SBUF first
                    dn_sb = sb.tile([1, TC], F32, tag="dn_sb", bufs=2)
                    nc.scalar.activation(dn_sb, py[HD : HD + 1, :], Act.Copy)
                    recip_f = sb.tile([1, TC], F32, tag="recip_f", bufs=2)
                    nc.vector.reciprocal_approx_fast(recip_f, dn_sb)
                    recip_b = sb.tile([1, TC], BF16, tag="recip_b", bufs=2)
                    nc.vector.tensor_copy(recip_b, recip_f)
                    # broadcast 1/denom across the 64 head-dim partitions on
                    # the (otherwise idle) gpsimd engine; keeps the final
                    # multiply to a single PSUM operand
                    rb = sb.tile([HD, TC], BF16, tag="rb", bufs=2)
                    nc.gpsimd.partition_broadcast(rb, recip_b)
                    nc.vector.tensor_mul(
                        yt_sb[prow : prow + 64, h // 2, q * TC : (q + 1) * TC],
                        py[0:HD, :],
                        rb,
                    )

                # build next chunk's x^T now: its transposes fill the PE gap
                # left by the last head's normalization chain
                if tcx + 1 < NTC:
                    xt = build_xt(tcx + 1)

                # ---- phase 3: partial projection for t-chunk = tcx ----
                for mt in range(8):
                    po = ps.tile([128, TC], F32, tag="mm512", bufs=4)
                    for cc in range(2):
                        nc.tensor.matmul(
                            po,
                            wp_sb[:, cc, mt * 128 : (mt + 1) * 128],
                            yt_sb[:, cc, tcx * TC : (tcx + 1) * TC],
                            start=(cc == 0),
                            stop=(cc == 1),
                        )
                    ot = sb.tile([128, TC], BF16, tag="ot", bufs=3)
                    nc.vector.tensor_copy(ot, po)
                    nc.sync.dma_start(
                        out=out_d[
                            mt * 128 : (mt + 1) * 128, tcx * TC : (tcx + 1) * TC
                        ],
                        in_=ot,
                    )

    nc.finalize()
    return nc


def _in_maps(x, W_attn, b_attn, W_proj):
    import ml_dtypes

    bf16 = ml_dtypes.bfloat16
    in_maps = []
    for core in range(8):
        b = core // HG
        hg = core % HG
        qs, ks, vs = hg * DG, C + hg * DG, 2 * C + hg * DG
        wqk = np.concatenate(
            [W_attn[:, qs : qs + DG], W_attn[:, ks : ks + DG]], axis=1
        )
        bqk = np.concatenate(
            [b_attn[qs : qs + DG], b_attn[ks : ks + DG]]
        ).reshape(2 * DG, 1)
        in_maps.append(
            {
                "xt": np.ascontiguousarray(x[b].T).astype(bf16),
                "wqk": np.ascontiguousarray(wqk).astype(bf16),
                "bqk": np.ascontiguousarray(bqk),
                "wv": np.ascontiguousarray(W_attn[:, vs : vs + DG]).astype(bf16),
                "bv": np.ascontiguousarray(b_attn[vs : vs + DG].reshape(1, DG)),
                "wp": np.ascontiguousarray(
                    W_proj[hg * DG : (hg + 1) * DG, :]
                ).astype(bf16),
            }
        )
    return in_maps


def _combine(results, b_proj):
    out = np.empty((B, T, C), dtype=np.float32)
    for b in range(B):
        acc = results[4 * b]["outt"].astype(np.float32)
        for hg in range(1, HG):
            acc = acc + results[4 * b + hg]["outt"].astype(np.float32)
        out[b] = acc.T + b_proj
    return out


def get_nc():
    if "nc" not in _cached:
        _cached["nc"] = _build()
    return _cached["nc"]


def kernel(x, W_attn, b_attn, W_proj, b_proj):
    from concourse.bass_utils import run_bass_kernel_spmd

    nc = get_nc()
    x = np.asarray(x, dtype=np.float32)
    W_attn = np.asarray(W_attn, dtype=np.float32)
    b_attn = np.asarray(b_attn, dtype=np.float32)
    W_proj = np.asarray(W_proj, dtype=np.float32)
    b_proj = np.asarray(b_proj, dtype=np.float32)

    in_maps = _in_maps(x, W_attn, b_attn, W_proj)
    r = run_bass_kernel_spmd(nc, in_maps, core_ids=list(range(8)), trace=False)
    return _combine(r.results, b_proj)
